# revision 14
# baseline (speedup 1.0000x reference)
"""Trainium2 Bass kernel for nn_Brain_connectomic_graph (GNN message passing).

Single tiny graph (N=100 nodes, E=2000 edges). Strategy: the whole network is
expressed as dense linear algebra on ONE NeuronCore and replicated across the
8 cores (data-parallel lanes with batch=1, per the sharding hint); core 0's
output is returned.

All floating-point math runs on device. The host only does layout packing:
  - transposes/concats of input tensors (pure data movement),
  - integer edge indices packed as f32 columns (one-hot encoding happens
    on-device via iota comparison),
  - pure constants (iota rows, strict-lower-triangular mask, identity, ones).

Graph ops are densified on device:
  - scatter-adds over edges -> one-hot matrices (built with DVE compares)
    contracted on the PE: A^T[src,dst] stacked for (unweighted | same-hemisphere
    weighted | full weighted) in one 16-chunk accumulation pass,
  - GCN normalization  -> row-scaling sandwich dis * ((A+I)^T @ (dis * XW)),
  - top-k(50)         -> rank via score comparison matrix (strict > plus
    index tie-break, matching jax.lax.top_k), permutation as one-hot matmul,
  - SAGPool / ChebConv / dense_diff_pool -> small matmuls + softmaxes.
"""

import numpy as np

N = 100
E = 2000
EP = 2048          # padded edges: 16 chunks x 128 partitions
NCH = 16
K1 = 50

# ---- inbuf column layout (f32 blob [128, C]) --------------------------------
_off = 0
def _nxt(w):
    global _off
    o = _off
    _off += w
    return o

O_XT    = _nxt(100)   # [100,100] x^T
O_SRC   = _nxt(16)    # [128,16]  src (f32, pad -1)
O_DST   = _nxt(16)    # [128,16]  dst (f32, pad -1)
O_EW    = _nxt(16)    # [128,16]  edge_attr (pad 0)
O_W1    = _nxt(128)   # [100,128] [Wl1 | Wr1]
O_W2    = _nxt(40)    # [64,40]   [Wl2 | Wr2]
O_WG    = _nxt(20)    # [20,20]   Wg1
O_WRR   = _nxt(1)     # [40,1]    [Wrel; Wroot]
O_WC    = _nxt(60)    # [20,60]   [Wc0 | Wc1 | Wc2] side by side
O_BC1   = _nxt(64)    # [100,64]  rows<50: bl1, rows>=50: br1
O_BC2   = _nxt(20)    # [100,20]  rows<50: bl2, rows>=50: br2
O_BG    = _nxt(20)    # [100,20]  bg1 broadcast
O_BCC   = _nxt(20)    # [100,20]  bc broadcast
O_BREL  = _nxt(1)     # [128,1]   brel broadcast
O_MKL   = _nxt(1)     # [128,1]   1.0 for p<50 else 0
O_MKR   = _nxt(1)     # [128,1]   1.0 for 50<=p<100 else 0
O_IOTA  = _nxt(100)   # [128,100] each row 0..99
O_IOT50 = _nxt(50)    # [128,50]  each row 0..49
O_TRIL  = _nxt(100)   # [100,100] strict lower: [n,m]=1 iff m<n
O_I100  = _nxt(100)   # [100,100] identity
O_ONES  = _nxt(100)   # [128,100] ones
C_COLS  = _off


def _split_multiwaits(bir: dict) -> dict:
    """This container's walrus accepts only ONE sync-wait per instruction.
    Insert single-wait NoOps (same engine, just before) for the extras."""
    for f in bir.get("functions", []):
        for bb in f.get("blocks", []):
            out = []
            for ins in bb.get("instructions", []):
                si = ins.get("sync_info")
                waits = (si or {}).get("on_wait") or []
                if len(waits) > 1:
                    for i, w in enumerate(waits[:-1]):
                        out.append({
                            "debug": ins.get("debug", 0),
                            "engine": ins["engine"],
                            "ins": [], "outs": [],
                            "name": f"{ins['name']}-w{i}",
                            "opcode": "NoOp",
                            "sync_info": {"on_wait": [w], "on_update": []},
                        })
                    si["on_wait"] = [waits[-1]]
                out.append(ins)
            bb["instructions"] = out
    return bir


def _build():
    import json
    import concourse.bass as bass
    import concourse.mybir as mybir
    import concourse.tile as tile

    f32 = mybir.dt.float32
    Alu = mybir.AluOpType
    Act = mybir.ActivationFunctionType
    AxX = mybir.AxisListType.X

    nc = bass.Bass("TRN2")
    in_d = nc.dram_tensor("inbuf", [128, C_COLS], f32, kind="ExternalInput")
    out_d = nc.dram_tensor("out", [K1, 20], f32, kind="ExternalOutput")

    with tile.TileContext(nc) as tc:
        with (
            tc.tile_pool(name="sb", bufs=1) as sb,
            tc.tile_pool(name="ps", bufs=1, space="PSUM") as ps,
        ):
            ib = sb.tile([128, C_COLS], f32, tag="ib", name="ib")
            nc.sync.dma_start(out=ib, in_=in_d.ap())

            def isl(off, w, p0=0, p1=128):
                return ib[p0:p1, off:off + w]

            XT   = isl(O_XT, 100, 0, 100)
            SRC  = isl(O_SRC, 16)
            DST  = isl(O_DST, 16)
            EW   = isl(O_EW, 16)
            W1   = isl(O_W1, 128, 0, 100)
            W2   = isl(O_W2, 40, 0, 64)
            WG   = isl(O_WG, 20, 0, 20)
            WRR  = isl(O_WRR, 1, 0, 40)
            WC0  = isl(O_WC, 20, 0, 20)
            WC1  = isl(O_WC + 20, 20, 0, 20)
            WC2  = isl(O_WC + 40, 20, 0, 20)
            BC1  = isl(O_BC1, 64, 0, 100)
            BC2  = isl(O_BC2, 20, 0, 100)
            BG   = isl(O_BG, 20, 0, 100)
            BCC  = isl(O_BCC, 20, 0, 100)
            BREL = isl(O_BREL, 1)
            MKL  = isl(O_MKL, 1, 0, 100)
            MKR  = isl(O_MKR, 1, 0, 100)
            IOTA = isl(O_IOTA, 100)
            IO50 = isl(O_IOT50, 50, 0, 100)
            TRIL = isl(O_TRIL, 100, 0, 100)
            I100 = isl(O_I100, 100, 0, 100)
            ONESR = isl(O_ONES, 100, 0, 1)     # [1,100] ones row
            ONESC = isl(O_ONES, 1, 0, 100)     # [100,1] ones col

            V = nc.vector
            S = nc.scalar
            T = nc.tensor

            # ---- edge masks -------------------------------------------------
            slt = sb.tile([128, 16], f32, tag="slt", name="slt")
            dlt = sb.tile([128, 16], f32, tag="dlt", name="dlt")
            same = sb.tile([128, 16], f32, tag="same", name="same")
            ews = sb.tile([128, 16], f32, tag="ews", name="ews")
            V.tensor_scalar(out=slt, in0=SRC, scalar1=50.0, scalar2=None, op0=Alu.is_lt)
            V.tensor_scalar(out=dlt, in0=DST, scalar1=50.0, scalar2=None, op0=Alu.is_lt)
            V.tensor_tensor(out=same, in0=slt, in1=dlt, op=Alu.is_equal)
            V.tensor_tensor(out=ews, in0=EW, in1=same, op=Alu.mult)

            # ---- one-hot edge matrices (all 16 chunks per DVE op) ----------
            # Ssrc[e, n] = [src_e == n]; R = [Sdst | Sdst*ew_same | Sdst*ew]
            ssrc = sb.tile([128, NCH * 100], f32, tag="ssrc", name="ssrc")
            rall = sb.tile([128, NCH * 300], f32, tag="rall", name="rall")
            ssrc3 = ssrc.rearrange("p (c j) -> p c j", c=NCH)
            rall3 = rall.rearrange("p (c j) -> p c j", c=NCH)
            iota_b = IOTA.unsqueeze(1).broadcast_to([128, NCH, 100])
            src_b = SRC.unsqueeze(2).broadcast_to([128, NCH, 100])
            dst_b = DST.unsqueeze(2).broadcast_to([128, NCH, 100])
            ews_b = ews[:].unsqueeze(2).broadcast_to([128, NCH, 100])
            ew_b = EW.unsqueeze(2).broadcast_to([128, NCH, 100])
            V.tensor_tensor(out=rall3[:, :, 0:100], in0=iota_b, in1=dst_b, op=Alu.is_equal)
            V.tensor_tensor(out=ssrc3[:, :, 0:100], in0=iota_b, in1=src_b, op=Alu.is_equal)
            V.tensor_tensor(out=rall3[:, :, 100:200], in0=rall3[:, :, 0:100], in1=ews_b, op=Alu.mult)
            V.tensor_tensor(out=rall3[:, :, 200:300], in0=rall3[:, :, 0:100], in1=ew_b, op=Alu.mult)

            # ---- adjacency stack: A_ps = [A1^T | Ac^T | Ag^T]  [100,300] ----
            a_ps = ps.tile([100, 300], f32, tag="acc", name="a_ps", bufs=1)
            for c in range(NCH):
                T.matmul(a_ps, ssrc3[:, c, :], rall3[:, c, :],
                         start=(c == 0), stop=(c == NCH - 1))

            a1t = sb.tile([100, 100], f32, tag="a1t", name="a1t")
            act = sb.tile([100, 100], f32, tag="act", name="act")
            agt = sb.tile([100, 100], f32, tag="agt", name="agt")
            V.tensor_copy(out=a1t, in_=a_ps[:, 0:100])
            V.tensor_tensor(out=act, in0=a_ps[:, 100:200], in1=I100, op=Alu.add)
            V.tensor_tensor(out=agt, in0=a_ps[:, 200:300], in1=I100, op=Alu.add)

            # ---- degrees + dis (GCN: deg+1 = rowsum(A+I)) -------------------
            mm = lambda shape, name: ps.tile(shape, f32, tag="mm", name=name, bufs=3)
            d1c = mm([100, 1], "d1c")
            T.matmul(d1c, act, ONESC)
            d1g = mm([100, 1], "d1g")
            T.matmul(d1g, agt, ONESC)
            disc_t = sb.tile([100, 1], f32, tag="disc", name="disc_t")
            disg_t = sb.tile([100, 1], f32, tag="disg", name="disg_t")
            S.activation(out=disc_t, in_=d1c, func=Act.Sqrt)
            V.reciprocal(out=disc_t, in_=disc_t)
            S.activation(out=disg_t, in_=d1g, func=Act.Sqrt)
            V.reciprocal(out=disg_t, in_=disg_t)
            # hemisphere-masked dis columns (partition slices must be 32-aligned,
            # so per-side selection is done by masked scalars instead)
            disL = sb.tile([100, 1], f32, tag="disL", name="disL")
            disR = sb.tile([100, 1], f32, tag="disR", name="disR")
            V.tensor_tensor(out=disL, in0=disc_t, in1=MKL, op=Alu.mult)
            V.tensor_tensor(out=disR, in0=disc_t, in1=MKR, op=Alu.mult)

            # ---- layer 1: h1 = lrelu(dis*((Ac+I)^T' @ (dis*xw_side)) + b) ---
            xw = mm([100, 128], "xw")
            T.matmul(xw, XT, W1)
            y1 = sb.tile([100, 64], f32, tag="y1", name="y1")
            V.tensor_scalar_mul(y1, xw[:, 0:64], disL)
            V.scalar_tensor_tensor(out=y1, in0=xw[:, 64:128], scalar=disR, in1=y1,
                                   op0=Alu.mult, op1=Alu.add)
            z1 = mm([100, 64], "z1")
            T.matmul(z1, act, y1)
            h1 = sb.tile([100, 64], f32, tag="h1", name="h1")
            V.scalar_tensor_tensor(out=h1, in0=z1, scalar=disc_t, in1=BC1, op0=Alu.mult, op1=Alu.add)
            V.scalar_tensor_tensor(out=h1, in0=h1, scalar=0.01, in1=h1, op0=Alu.mult, op1=Alu.max)

            # ---- layer 2 ----------------------------------------------------
            h1t_p = mm([64, 100], "h1t_p")
            T.transpose(h1t_p, h1, I100)
            h1t = sb.tile([64, 100], f32, tag="h1t", name="h1t")
            V.tensor_copy(out=h1t, in_=h1t_p)
            xw2 = mm([100, 40], "xw2")
            T.matmul(xw2, h1t, W2)
            y2 = sb.tile([100, 20], f32, tag="y2", name="y2")
            V.tensor_scalar_mul(y2, xw2[:, 0:20], disL)
            V.scalar_tensor_tensor(out=y2, in0=xw2[:, 20:40], scalar=disR, in1=y2,
                                   op0=Alu.mult, op1=Alu.add)
            z2 = mm([100, 20], "z2")
            T.matmul(z2, act, y2)
            h2a = sb.tile([100, 20], f32, tag="h2a", name="h2a")
            V.scalar_tensor_tensor(out=h2a, in0=z2, scalar=disc_t, in1=BC2, op0=Alu.mult, op1=Alu.add)
            V.scalar_tensor_tensor(out=h2a, in0=h2a, scalar=0.01, in1=h2a, op0=Alu.mult, op1=Alu.max)

            # ---- global GCN layer ------------------------------------------
            h2at_p = mm([20, 100], "h2at_p")
            T.transpose(h2at_p, h2a, I100)
            h2at = sb.tile([20, 100], f32, tag="h2at", name="h2at")
            V.tensor_copy(out=h2at, in_=h2at_p)
            xwg = mm([100, 20], "xwg")
            T.matmul(xwg, h2at, WG)
            yg = sb.tile([100, 20], f32, tag="yg", name="yg")
            V.tensor_scalar_mul(yg, xwg, disg_t)
            zg = mm([100, 20], "zg")
            T.matmul(zg, agt, yg)
            h2 = sb.tile([100, 20], f32, tag="h2", name="h2")
            V.scalar_tensor_tensor(out=h2, in0=zg, scalar=disg_t, in1=BG, op0=Alu.mult, op1=Alu.add)
            V.scalar_tensor_tensor(out=h2, in0=h2, scalar=0.01, in1=h2, op0=Alu.mult, op1=Alu.max)

            # ---- SAGPool score ---------------------------------------------
            agg = mm([100, 20], "agg")
            T.matmul(agg, a1t, h2)
            cc = sb.tile([100, 40], f32, tag="cc", name="cc")
            V.tensor_copy(out=cc[:, 0:20], in_=agg)
            S.copy(out=cc[:, 20:40], in_=h2)
            cct_p = mm([40, 100], "cct_p")
            T.transpose(cct_p, cc, I100)
            cct = sb.tile([40, 100], f32, tag="cct", name="cct")
            V.tensor_copy(out=cct, in_=cct_p)
            sc_p = mm([100, 1], "sc_p")
            T.matmul(sc_p, cct, WRR)          # score (without brel; rank-invariant)
            score = sb.tile([100, 1], f32, tag="score", name="score")
            V.tensor_copy(out=score, in_=sc_p)

            # ---- rank / top-k as matrices ----------------------------------
            srow_p = mm([1, 100], "srow_p")
            T.transpose(srow_p, score, I100)
            srow = sb.tile([1, 100], f32, tag="srow", name="srow")
            V.tensor_copy(out=srow, in_=srow_p)
            srep = ps.tile([100, 100], f32, tag="rep", name="srep", bufs=2)
            T.matmul(srep, ONESR, srow)       # srep[n,m] = score[m]
            t2 = sb.tile([100, 100], f32, tag="t2", name="t2")
            V.scalar_tensor_tensor(out=t2, in0=srep, scalar=score, in1=TRIL, op0=Alu.is_equal, op1=Alu.mult)
            csum = sb.tile([100, 100], f32, tag="csum", name="csum")
            rank = sb.tile([100, 1], f32, tag="rank", name="rank")
            V.scalar_tensor_tensor(out=csum, in0=srep, scalar=score, in1=t2, op0=Alu.is_gt, op1=Alu.add,
                                   accum_out=rank)
            kept = sb.tile([100, 1], f32, tag="kept", name="kept")
            V.tensor_scalar(out=kept, in0=rank, scalar1=49.5, scalar2=None, op0=Alu.is_lt)
            pit = sb.tile([100, 50], f32, tag="pit", name="pit")
            V.tensor_scalar(out=pit, in0=IO50, scalar1=rank, scalar2=None, op0=Alu.is_equal)

            krow_p = mm([1, 100], "krow_p")
            T.transpose(krow_p, kept, I100)
            krow = sb.tile([1, 100], f32, tag="krow", name="krow")
            V.tensor_copy(out=krow, in_=krow_p)
            krep = ps.tile([100, 100], f32, tag="rep", name="krep", bufs=2)
            T.matmul(krep, ONESR, krow)       # krep[n,m] = kept[m]
            kscr = sb.tile([100, 100], f32, tag="kscr", name="kscr")
            srank = sb.tile([100, 1], f32, tag="srank", name="srank")
            V.tensor_tensor(out=kscr, in0=krep, in1=TRIL, op=Alu.mult)
            V.tensor_reduce(out=srank, in_=kscr, axis=AxX, op=Alu.add)
            gat = sb.tile([100, 50], f32, tag="gat", name="gat")
            V.tensor_scalar(out=gat, in0=IO50, scalar1=srank, scalar2=None, op0=Alu.is_equal)
            V.tensor_scalar_mul(gat, gat, kept)

            # ---- pooled adjacency Atil = Pi @ A1 @ Pi^T --------------------
            m1 = mm([100, 50], "m1")
            T.matmul(m1, a1t, pit)
            m1s = sb.tile([100, 50], f32, tag="m1s", name="m1s")
            V.tensor_copy(out=m1s, in_=m1)
            atil = mm([50, 50], "atil")
            T.matmul(atil, pit, m1s)
            degc = sb.tile([50, 1], f32, tag="degc", name="degc")
            V.tensor_reduce(out=degc, in_=atil, axis=AxX, op=Alu.add)
            atils = sb.tile([50, 50], f32, tag="atils", name="atils")
            V.tensor_copy(out=atils, in_=atil)
            atilt_p = mm([50, 50], "atilt_p")
            T.transpose(atilt_p, atils, I100[0:50, 0:50])
            atilt = sb.tile([50, 50], f32, tag="atilt", name="atilt")
            V.tensor_copy(out=atilt, in_=atilt_p)

            # disč = where(deg>0, rsqrt(max(deg,1e-12)), 0)
            dm = sb.tile([50, 1], f32, tag="dm", name="dm")
            V.tensor_scalar(out=dm, in0=degc, scalar1=1e-12, scalar2=None, op0=Alu.max)
            S.activation(out=dm, in_=dm, func=Act.Sqrt)
            V.reciprocal(out=dm, in_=dm)
            m0 = sb.tile([50, 1], f32, tag="m0", name="m0")
            V.tensor_scalar(out=m0, in0=degc, scalar1=0.0, scalar2=None, op0=Alu.is_gt)
            disch = sb.tile([50, 1], f32, tag="disch", name="disch")
            V.tensor_tensor(out=disch, in0=dm, in1=m0, op=Alu.mult)
            # extended to 100 rows (0 beyond 50) so Tx ops run at partition 0
            dise = sb.tile([100, 1], f32, tag="dise", name="dise")
            V.memset(dise, 0.0)
            V.tensor_copy(out=dise[0:50, :], in_=disch)
            ndis = sb.tile([100, 1], f32, tag="ndis", name="ndis")
            V.tensor_scalar_mul(ndis, dise, -1.0)
            n2dis = sb.tile([100, 1], f32, tag="n2dis", name="n2dis")
            V.tensor_scalar_mul(n2dis, dise, -2.0)
            # Atil^T padded to [50,100] so matmul M=100 (rows >=50 produce 0)
            atx = sb.tile([50, 100], f32, tag="atx", name="atx")
            V.memset(atx, 0.0)
            V.tensor_copy(out=atx[:, 0:50], in_=atilt)

            # ---- Cheb Tx1 / Tx2 --------------------------------------------
            y1c = sb.tile([50, 20], f32, tag="y1c", name="y1c")
            V.tensor_scalar_mul(y1c, h2[0:50, :], disch)
            tx1p = mm([100, 20], "tx1p")
            T.matmul(tx1p, atx, y1c)
            tx1f = sb.tile([100, 20], f32, tag="tx1f", name="tx1f")
            V.tensor_scalar_mul(tx1f, tx1p, ndis)      # rows>=50 -> 0
            y2c = sb.tile([50, 20], f32, tag="y2c", name="y2c")
            V.tensor_scalar_mul(y2c, tx1f[0:50, :], disch)
            tx2p = mm([100, 20], "tx2p")
            T.matmul(tx2p, atx, y2c)
            tx2f = sb.tile([100, 20], f32, tag="tx2f", name="tx2f")
            # rows<50: -2dis*t - h2 ; rows>=50: 0 - h2  (= -Tx0, as required)
            V.scalar_tensor_tensor(out=tx2f, in0=tx2p, scalar=n2dis, in1=h2,
                                   op0=Alu.mult, op1=Alu.subtract)

            # ---- s_raw = h2@Wc0 + Tx1@Wc1 + Tx2@Wc2 + bc --------------------
            sraw_p = mm([100, 20], "sraw_p")
            for i, (tq, wc) in enumerate(((h2, WC0), (tx1f, WC1), (tx2f, WC2))):
                tq_tp = mm([20, 100], f"tq_tp{i}")
                T.transpose(tq_tp, tq, I100)
                tq_ts = sb.tile([20, 100], f32, tag=f"tqts{i}", name=f"tqts{i}")
                V.tensor_copy(out=tq_ts, in_=tq_tp)
                T.matmul(sraw_p, tq_ts, wc, start=(i == 0), stop=(i == 2))
            sraw = sb.tile([100, 20], f32, tag="sraw", name="sraw")
            V.tensor_tensor(out=sraw, in0=sraw_p, in1=BCC, op=Alu.add)

            # ---- double softmax --------------------------------------------
            def softmax(dst_t, src_t, idx):
                negm = sb.tile([100, 1], f32, tag=f"negm{idx}", name=f"negm{idx}")
                V.tensor_reduce(out=negm, in_=src_t, axis=AxX, op=Alu.max, negate=True)
                ex = sb.tile([100, 20], f32, tag=f"ex{idx}", name=f"ex{idx}")
                S.activation(out=ex, in_=src_t, func=Act.Exp, bias=negm, scale=1.0)
                ssum = sb.tile([100, 1], f32, tag=f"ssum{idx}", name=f"ssum{idx}")
                V.tensor_reduce(out=ssum, in_=ex, axis=AxX, op=Alu.add)
                V.reciprocal(out=ssum, in_=ssum)
                V.tensor_scalar_mul(dst_t, ex, ssum)

            ass = sb.tile([100, 20], f32, tag="ass", name="ass")
            softmax(ass, sraw, 0)
            s2 = sb.tile([100, 20], f32, tag="s2", name="s2")
            softmax(s2, ass, 1)

            # ---- diff-pool + output ----------------------------------------
            hc_p = mm([20, 20], "hc_p")
            T.matmul(hc_p, s2, h2)            # H_coarse = s2^T @ h2
            hc = sb.tile([20, 20], f32, tag="hc", name="hc")
            V.tensor_copy(out=hc, in_=hc_p)
            asst_p = mm([20, 100], "asst_p")
            T.transpose(asst_p, ass, I100)
            asst = sb.tile([20, 100], f32, tag="asst", name="asst")
            V.tensor_copy(out=asst, in_=asst_p)
            asc = mm([100, 20], "asc")
            T.matmul(asc, asst, hc)           # ass @ H_coarse
            ascs = sb.tile([100, 20], f32, tag="ascs", name="ascs")
            V.tensor_copy(out=ascs, in_=asc)
            g_p = mm([50, 20], "g_p")
            T.matmul(g_p, gat, ascs)          # inter @ H_coarse (rows perm order)

            p1 = mm([50, 20], "p1")
            T.matmul(p1, pit, h2)             # h2[perm]
            ts_p = mm([50, 1], "ts_p")
            T.matmul(ts_p, pit, score)        # top scores (without brel)
            th = sb.tile([50, 1], f32, tag="th", name="th")
            S.activation(out=th, in_=ts_p, func=Act.Tanh, bias=BREL[0:50, :], scale=1.0)
            gs = sb.tile([50, 20], f32, tag="gs", name="gs")
            V.tensor_copy(out=gs, in_=g_p)
            outv = sb.tile([50, 20], f32, tag="outv", name="outv")
            V.scalar_tensor_tensor(out=outv, in0=p1, scalar=th, in1=gs, op0=Alu.mult, op1=Alu.add)
            nc.sync.dma_start(out=out_d.ap(), in_=outv)

    # walrus single-wait workaround
    orig = nc.to_json_bytes
    def patched(*a, **k):
        import json as _json
        return _json.dumps(_split_multiwaits(_json.loads(orig(*a, **k)))).encode()
    nc.to_json_bytes = patched
    return nc


def _pack(inputs) -> np.ndarray:
    f = lambda k: np.asarray(inputs[k], dtype=np.float32)
    blob = np.zeros((128, C_COLS), dtype=np.float32)

    x = f("x")
    blob[0:100, O_XT:O_XT + 100] = x.T

    ei = np.asarray(inputs["edge_index"]).astype(np.int64)
    src = np.full(EP, -1.0, np.float32); src[:E] = ei[0]
    dst = np.full(EP, -1.0, np.float32); dst[:E] = ei[1]
    ew = np.zeros(EP, np.float32); ew[:E] = f("edge_attr")
    # column-chunk layout: element (p, c) = edge c*128+p
    blob[:, O_SRC:O_SRC + 16] = src.reshape(NCH, 128).T
    blob[:, O_DST:O_DST + 16] = dst.reshape(NCH, 128).T
    blob[:, O_EW:O_EW + 16] = ew.reshape(NCH, 128).T

    blob[0:100, O_W1:O_W1 + 64] = f("Wl1")
    blob[0:100, O_W1 + 64:O_W1 + 128] = f("Wr1")
    blob[0:64, O_W2:O_W2 + 20] = f("Wl2")
    blob[0:64, O_W2 + 20:O_W2 + 40] = f("Wr2")
    blob[0:20, O_WG:O_WG + 20] = f("Wg1")
    blob[0:20, O_WRR] = f("Wrel")[:, 0]
    blob[20:40, O_WRR] = f("Wroot")[:, 0]
    blob[0:20, O_WC:O_WC + 20] = f("Wc0")
    blob[0:20, O_WC + 20:O_WC + 40] = f("Wc1")
    blob[0:20, O_WC + 40:O_WC + 60] = f("Wc2")
    blob[0:50, O_BC1:O_BC1 + 64] = f("bl1")
    blob[50:100, O_BC1:O_BC1 + 64] = f("br1")
    blob[0:50, O_BC2:O_BC2 + 20] = f("bl2")
    blob[50:100, O_BC2:O_BC2 + 20] = f("br2")
    blob[0:100, O_BG:O_BG + 20] = f("bg1")
    blob[0:100, O_BCC:O_BCC + 20] = f("bc")
    blob[:, O_BREL] = f("brel")[0]
    blob[0:50, O_MKL] = 1.0
    blob[50:100, O_MKR] = 1.0
    blob[:, O_IOTA:O_IOTA + 100] = np.arange(100, dtype=np.float32)
    blob[:, O_IOT50:O_IOT50 + 50] = np.arange(50, dtype=np.float32)
    n_i = np.arange(100)
    blob[0:100, O_TRIL:O_TRIL + 100] = (n_i[None, :] < n_i[:, None]).astype(np.float32)
    blob[0:100, O_I100:O_I100 + 100] = np.eye(100, dtype=np.float32)
    blob[:, O_ONES:O_ONES + 100] = 1.0
    return blob


_NC = None

def _get_nc():
    global _NC
    if _NC is None:
        _NC = _build()
    return _NC


def run(inputs, trace=False):
    from concourse.bass_utils import run_bass_kernel_spmd
    nc = _get_nc()
    blob = _pack(inputs)
    in_maps = [{"inbuf": blob} for _ in range(8)]
    res = run_bass_kernel_spmd(nc, in_maps, list(range(8)), trace=trace)
    out = np.asarray(res.results[0]["out"], dtype=np.float32).reshape(1, K1 * 20)
    return out, res


def kernel(**inputs) -> np.ndarray:
    out, _ = run(inputs)
    return out


# revision 39
# speedup vs baseline: 1.2395x; 1.2395x over previous
"""Trainium2 Bass kernel for nn_Brain_connectomic_graph (GNN message passing).

Single tiny graph (N=100 nodes, E=2000 edges). Strategy: the whole network is
expressed as dense linear algebra on ONE NeuronCore and replicated across the
8 cores (data-parallel lanes with batch=1, per the sharding hint); core 0's
output is returned.

All floating-point math runs on device. The host only does layout packing:
  - transposes/concats of input tensors (pure data movement),
  - integer edge indices packed as f32 columns (one-hot encoding happens
    on-device via iota comparison),
  - pure constants (iota rows, triangular masks, identity, ones).

Graph ops are densified on device:
  - scatter-adds over edges -> one-hot matrices (DVE compares, pipelined in
    4 chunk-groups with the weighted variants on GpSimd) contracted on the
    PE: A^T stacked for (unweighted | same-hemisphere | full weighted),
  - GCN normalization  -> row-scaling sandwich dis * ((A+I)^T' @ (dis * XW)),
  - top-k(50)         -> rank via score comparison matrix (strict > plus
    index tie-break, matching jax.lax.top_k), permutation as one-hot matmul,
  - SAGPool / ChebConv / dense_diff_pool -> small matmuls + softmaxes.
"""

import numpy as np

N = 100
E = 2000
EP = 2048          # padded edges: 16 chunks x 128 partitions
NCH = 16
K1 = 50

# ---- inbuf column layout (f32 blob [128, C]) --------------------------------
# Ordered by when the kernel needs the data; loaded as 3 parallel DMAs.
_off = 0
def _nxt(w):
    global _off
    o = _off
    _off += w
    return o

# DMA group A (own DRAM tensor, contiguous): edge data
O_SRC   = _nxt(16)    # [128,16]  src (f32, pad -1)
O_DST   = _nxt(16)    # [128,16]  dst (f32, pad -1)
O_EW    = _nxt(16)    # [128,16]  edge_attr (pad 0)
C_DMA_A = _off
# DMA group B: first matmul operands
O_XT    = _nxt(100)   # [100,100] x^T
O_W1    = _nxt(128)   # [100,128] [Wl1 | Wr1]
C_DMA_B = _off
# DMA group C: everything else
O_W2    = _nxt(40)    # [64,40]   [Wl2 | Wr2]
O_WG    = _nxt(20)    # [20,20]   Wg1
O_WREL  = _nxt(1)     # [20,1]    Wrel
O_WROOT = _nxt(1)     # [20,1]    Wroot
O_WC    = _nxt(60)    # [20,60]   [Wc0 | Wc1 | Wc2]
O_BC1   = _nxt(64)    # [100,64]  rows<50: bl1, rows>=50: br1
O_BC2   = _nxt(20)    # [100,20]  rows<50: bl2, rows>=50: br2
O_BG    = _nxt(20)    # [100,20]  bg1 broadcast
O_BCC   = _nxt(20)    # [100,20]  bc broadcast
O_BREL  = _nxt(1)     # [128,1]   brel broadcast
O_MKL   = _nxt(1)     # [128,1]   1.0 for p<50 else 0
O_MKR   = _nxt(1)     # [128,1]   1.0 for 50<=p<100 else 0
O_MBD   = _nxt(100)   # [100,100] block mask: [b,a]=1 iff (b<50)==(a<50)
C_COLS  = _off
# Pure constants (iota / identity / tril / triu / ones) are generated
# on-device by GpSimd during the DMA window.


def _split_multiwaits(bir: dict) -> dict:
    """This container's walrus accepts only ONE sync-wait per instruction.
    Insert single-wait NoOps (same engine, just before) for the extras."""
    for f in bir.get("functions", []):
        for bb in f.get("blocks", []):
            out = []
            for ins in bb.get("instructions", []):
                si = ins.get("sync_info")
                waits = (si or {}).get("on_wait") or []
                if len(waits) > 1:
                    for i, w in enumerate(waits[:-1]):
                        out.append({
                            "debug": ins.get("debug", 0),
                            "engine": ins["engine"],
                            "ins": [], "outs": [],
                            "name": f"{ins['name']}-w{i}",
                            "opcode": "NoOp",
                            "sync_info": {"on_wait": [w], "on_update": []},
                        })
                    si["on_wait"] = [waits[-1]]
                out.append(ins)
            bb["instructions"] = out
    return bir


def _build():
    import concourse.bass as bass
    import concourse.mybir as mybir
    import concourse.tile as tile

    f32 = mybir.dt.float32
    Alu = mybir.AluOpType
    Act = mybir.ActivationFunctionType
    AxX = mybir.AxisListType.X

    nc = bass.Bass("TRN2")
    in_a = nc.dram_tensor("inbufA", [128, C_DMA_A], f32, kind="ExternalInput")
    in_b = nc.dram_tensor("inbufB", [128, C_DMA_B - C_DMA_A], f32, kind="ExternalInput")
    in_c = nc.dram_tensor("inbufC", [128, C_COLS - C_DMA_B], f32, kind="ExternalInput")
    out_d = nc.dram_tensor("out", [K1, 20], f32, kind="ExternalOutput")

    with tile.TileContext(nc) as tc:
        with (
            tc.tile_pool(name="sb", bufs=1) as sb,
            tc.tile_pool(name="ps", bufs=1, space="PSUM") as ps,
        ):
            ib = sb.tile([128, C_COLS], f32, tag="ib", name="ib")
            nc.sync.dma_start(out=ib[:, 0:C_DMA_A], in_=in_a.ap())
            nc.sync.dma_start(out=ib[:, C_DMA_A:C_DMA_B], in_=in_b.ap())
            nc.sync.dma_start(out=ib[:, C_DMA_B:C_COLS], in_=in_c.ap())

            def isl(off, w, p0=0, p1=128):
                return ib[p0:p1, off:off + w]

            # ---- on-device constants (GpSimd, runs during the DMAs) ---------
            iota_i = sb.tile([128, 100], mybir.dt.int32, tag="iota_i", name="iota_i")
            nc.gpsimd.iota(iota_i, pattern=[[1, 100]], base=0, channel_multiplier=0)
            iota_t = sb.tile([128, 100], f32, tag="iota_t", name="iota_t")
            nc.gpsimd.tensor_copy(out=iota_t, in_=iota_i)
            i100_t = sb.tile([100, 100], f32, tag="i100_t", name="i100_t")
            nc.gpsimd.memset(i100_t, 0.0)
            nc.gpsimd.affine_select(out=i100_t, in_=i100_t, compare_op=mybir.AluOpType.not_equal,
                                    fill=1.0, base=0, pattern=[[-1, 100]], channel_multiplier=1)
            tril_t = sb.tile([100, 100], f32, tag="tril_t", name="tril_t")
            nc.gpsimd.memset(tril_t, 1.0)
            nc.gpsimd.affine_select(out=tril_t, in_=tril_t, compare_op=mybir.AluOpType.is_gt,
                                    fill=0.0, base=0, pattern=[[-1, 100]], channel_multiplier=1)
            triu_t = sb.tile([100, 100], f32, tag="triu_t", name="triu_t")
            nc.gpsimd.memset(triu_t, 1.0)
            nc.gpsimd.affine_select(out=triu_t, in_=triu_t, compare_op=mybir.AluOpType.is_gt,
                                    fill=0.0, base=0, pattern=[[1, 100]], channel_multiplier=-1)
            ones_t = sb.tile([128, 100], f32, tag="ones_t", name="ones_t")
            nc.gpsimd.memset(ones_t, 1.0)

            XT   = isl(O_XT, 100, 0, 100)
            SRC  = isl(O_SRC, 16)
            DST  = isl(O_DST, 16)
            EW   = isl(O_EW, 16)
            W1   = isl(O_W1, 128, 0, 100)
            W2   = isl(O_W2, 40, 0, 64)
            WG   = isl(O_WG, 20, 0, 20)
            WRR2 = isl(O_WREL, 2, 0, 20)      # [Wrel | Wroot]
            WC0  = isl(O_WC, 20, 0, 20)
            WC1  = isl(O_WC + 20, 20, 0, 20)
            WC2  = isl(O_WC + 40, 20, 0, 20)
            BC1  = isl(O_BC1, 64, 0, 100)
            BC2  = isl(O_BC2, 20, 0, 100)
            BG   = isl(O_BG, 20, 0, 100)
            BCC  = isl(O_BCC, 20, 0, 100)
            BREL = isl(O_BREL, 1)
            MKL  = isl(O_MKL, 1, 0, 100)
            MKR  = isl(O_MKR, 1, 0, 100)
            MBD  = isl(O_MBD, 100, 0, 100)
            IOTA = iota_t[:, :]
            IO50 = iota_t[0:100, 0:50]
            TRIL = tril_t[:, :]
            TRIU = triu_t[:, :]
            I100 = i100_t[:, :]
            ONESR = ones_t[0:1, :]             # [1,100] ones row
            ONESC = ones_t[0:100, 0:1]         # [100,1] ones col

            V = nc.vector
            S = nc.scalar
            P = nc.gpsimd
            T = nc.tensor
            mm = lambda shape, name: ps.tile(shape, f32, tag="mm", name=name, bufs=3)

            # ---- ACT table prewarm (Exp/Tanh tables load during prologue) ---
            scr = sb.tile([1, 1], f32, tag="scr", name="scr")
            V.memset(scr, 0.0)
            S.activation(out=scr, in_=scr, func=Act.Exp)
            S.activation(out=scr, in_=scr, func=Act.Tanh)
            S.activation(out=scr, in_=scr, func=Act.Sqrt)

            # ---- PE warmup: dummy matmuls on ones (HAM needs ~4us busy),
            # then xw (only needs DMA group B) --------------------------------
            warm = ps.tile([100, 300], f32, tag="warm", name="warm", bufs=1)
            ones_w = ones_t[:, :].unsqueeze(1).broadcast_to([128, 3, 100])
            for _ in range(4):
                T.matmul(warm, ones_t[:, :], ones_w)
            xw = mm([100, 128], "xw")
            T.matmul(xw, XT, W1)

            # ---- one-hot edge matrices, pipelined in 4 chunk-groups --------
            # Ssrc[e,n] = [src_e == n]; R = [Sdst | Sdst*ew].
            # A_c (same-hemisphere) is NOT built from edges: it equals the
            # block mask applied to A_g, so only 3 one-hot tensors are needed.
            ssrc = sb.tile([128, NCH * 100], f32, tag="ssrc", name="ssrc")
            rall = sb.tile([128, NCH * 200], f32, tag="rall", name="rall")
            ssrc3 = ssrc.rearrange("p (c j) -> p c j", c=NCH)
            rall3 = rall.rearrange("p (c j) -> p c j", c=NCH)
            a_ps = ps.tile([100, 200], f32, tag="acc", name="a_ps", bufs=1)
            GRP = 4
            for g in range(0, NCH, GRP):
                gs_, ge_ = g, g + GRP
                iota_b = IOTA.unsqueeze(1).broadcast_to([128, GRP, 100])
                src_b = SRC[:, gs_:ge_].unsqueeze(2).broadcast_to([128, GRP, 100])
                dst_b = DST[:, gs_:ge_].unsqueeze(2).broadcast_to([128, GRP, 100])
                ew_b = EW[:, gs_:ge_].unsqueeze(2).broadcast_to([128, GRP, 100])
                V.tensor_tensor(out=rall3[:, gs_:ge_, 0:100], in0=iota_b, in1=dst_b, op=Alu.is_equal)
                V.tensor_tensor(out=ssrc3[:, gs_:ge_, 0:100], in0=iota_b, in1=src_b, op=Alu.is_equal)
                P.tensor_tensor(out=rall3[:, gs_:ge_, 100:200], in0=rall3[:, gs_:ge_, 0:100], in1=ew_b, op=Alu.mult)
                for c in range(gs_, ge_):
                    T.matmul(a_ps, ssrc3[:, c, :], rall3[:, c, :],
                             start=(c == 0), stop=(c == NCH - 1))

            a1t = sb.tile([100, 100], f32, tag="a1t", name="a1t")
            act = sb.tile([100, 100], f32, tag="act", name="act")
            agt = sb.tile([100, 100], f32, tag="agt", name="agt")
            V.tensor_copy(out=a1t, in_=a_ps[:, 0:100])
            V.tensor_tensor(out=act, in0=a_ps[:, 100:200], in1=MBD, op=Alu.mult)
            V.tensor_tensor(out=act, in0=act, in1=I100, op=Alu.add)
            V.tensor_tensor(out=agt, in0=a_ps[:, 100:200], in1=I100, op=Alu.add)

            # ---- degrees + dis (GCN: deg+1 = rowsum(A+I)) -------------------
            d1c = mm([100, 1], "d1c")
            T.matmul(d1c, act, ONESC)
            d1g = mm([100, 1], "d1g")
            T.matmul(d1g, agt, ONESC)
            disc_t = sb.tile([100, 1], f32, tag="disc", name="disc_t")
            disg_t = sb.tile([100, 1], f32, tag="disg", name="disg_t")
            S.activation(out=disc_t, in_=d1c, func=Act.Sqrt)
            V.reciprocal(out=disc_t, in_=disc_t)
            S.activation(out=disg_t, in_=d1g, func=Act.Sqrt)
            V.reciprocal(out=disg_t, in_=disg_t)
            # hemisphere-masked dis columns (partition slices must be 32-aligned,
            # so per-side selection is done by masked scalars instead)
            disL = sb.tile([100, 1], f32, tag="disL", name="disL")
            disR = sb.tile([100, 1], f32, tag="disR", name="disR")
            V.tensor_tensor(out=disL, in0=disc_t, in1=MKL, op=Alu.mult)
            V.tensor_tensor(out=disR, in0=disc_t, in1=MKR, op=Alu.mult)

            # ---- layer 1: h1 = lrelu(dis*((Ac+I)^T' @ (dis*xw_side)) + b) ---
            y1 = sb.tile([100, 64], f32, tag="y1", name="y1")
            V.tensor_scalar_mul(y1, xw[:, 0:64], disL)
            V.scalar_tensor_tensor(out=y1, in0=xw[:, 64:128], scalar=disR, in1=y1,
                                   op0=Alu.mult, op1=Alu.add)
            z1 = mm([100, 64], "z1")
            T.matmul(z1, act, y1)
            h1 = sb.tile([100, 64], f32, tag="h1", name="h1")
            V.scalar_tensor_tensor(out=h1, in0=z1, scalar=disc_t, in1=BC1, op0=Alu.mult, op1=Alu.add)
            V.scalar_tensor_tensor(out=h1, in0=h1, scalar=0.01, in1=h1, op0=Alu.mult, op1=Alu.max)

            # ---- layer 2 ----------------------------------------------------
            h1t_p = mm([64, 100], "h1t_p")
            T.transpose(h1t_p, h1, I100)
            h1t = sb.tile([64, 100], f32, tag="h1t", name="h1t")
            V.tensor_copy(out=h1t, in_=h1t_p)
            xw2 = mm([100, 40], "xw2")
            T.matmul(xw2, h1t, W2)
            y2 = sb.tile([100, 20], f32, tag="y2", name="y2")
            V.tensor_scalar_mul(y2, xw2[:, 0:20], disL)
            V.scalar_tensor_tensor(out=y2, in0=xw2[:, 20:40], scalar=disR, in1=y2,
                                   op0=Alu.mult, op1=Alu.add)
            z2 = mm([100, 20], "z2")
            T.matmul(z2, act, y2)
            h2a = sb.tile([100, 20], f32, tag="h2a", name="h2a")
            V.scalar_tensor_tensor(out=h2a, in0=z2, scalar=disc_t, in1=BC2, op0=Alu.mult, op1=Alu.add)
            V.scalar_tensor_tensor(out=h2a, in0=h2a, scalar=0.01, in1=h2a, op0=Alu.mult, op1=Alu.max)

            # ---- global GCN layer ------------------------------------------
            h2at_p = mm([20, 100], "h2at_p")
            T.transpose(h2at_p, h2a, I100)
            h2at = sb.tile([20, 100], f32, tag="h2at", name="h2at")
            V.tensor_copy(out=h2at, in_=h2at_p)
            xwg = mm([100, 20], "xwg")
            T.matmul(xwg, h2at, WG)
            yg = sb.tile([100, 20], f32, tag="yg", name="yg")
            V.tensor_scalar_mul(yg, xwg, disg_t)
            zg = mm([100, 20], "zg")
            T.matmul(zg, agt, yg)
            # h2 lives in cols 0:20 of h2x; the SAG score joins as col 20 so
            # one matmul later produces both h2[perm] and score[perm].
            h2x = sb.tile([100, 21], f32, tag="h2x", name="h2x")
            h2 = h2x[:, 0:20]
            score = h2x[:, 20:21]
            V.scalar_tensor_tensor(out=h2, in0=zg, scalar=disg_t, in1=BG, op0=Alu.mult, op1=Alu.add)
            V.scalar_tensor_tensor(out=h2, in0=h2, scalar=0.01, in1=h2, op0=Alu.mult, op1=Alu.max)
            # h2^T, reused by the score matmuls and s_raw stage
            h2t_p = mm([20, 100], "h2t_p")
            T.transpose(h2t_p, h2, I100)
            h2t = sb.tile([20, 100], f32, tag="h2t", name="h2t")
            V.tensor_copy(out=h2t, in_=h2t_p)

            # ---- SAGPool score = A1@(h2@Wrel) + h2@Wroot  (brel in tanh) ----
            hw = mm([100, 2], "hw")
            T.matmul(hw, h2t, WRR2)           # [h2@Wrel | h2@Wroot]
            hw_sb = sb.tile([100, 2], f32, tag="hw_sb", name="hw_sb")
            V.tensor_copy(out=hw_sb, in_=hw)
            sc_p = mm([100, 1], "sc_p")
            T.matmul(sc_p, a1t, hw_sb[:, 0:1])
            V.tensor_tensor(out=score, in0=sc_p, in1=hw_sb[:, 1:2], op=Alu.add)

            # ---- rank / top-k as matrices ----------------------------------
            # score row MUST be bit-identical to the score column (the rank
            # comparisons mix both); a PE transpose preserves bits, a separate
            # matmul accumulation order does not.
            srow_p = mm([1, 100], "srow_p")
            T.transpose(srow_p, score, I100)
            srow = sb.tile([1, 100], f32, tag="srow", name="srow")
            V.tensor_copy(out=srow, in_=srow_p)
            srep = ps.tile([100, 100], f32, tag="rep", name="srep", bufs=1)
            T.matmul(srep, ONESR, srow)       # srep[n,m] = score[m]
            t2 = sb.tile([100, 100], f32, tag="t2", name="t2")
            V.scalar_tensor_tensor(out=t2, in0=srep, scalar=score, in1=TRIL, op0=Alu.is_equal, op1=Alu.mult)
            csum = sb.tile([100, 100], f32, tag="csum", name="csum")
            rank = sb.tile([100, 1], f32, tag="rank", name="rank")
            V.scalar_tensor_tensor(out=csum, in0=srep, scalar=score, in1=t2, op0=Alu.is_gt, op1=Alu.add,
                                   accum_out=rank)
            kept = sb.tile([100, 1], f32, tag="kept", name="kept")
            V.tensor_scalar(out=kept, in0=rank, scalar1=49.5, scalar2=None, op0=Alu.is_lt)
            pit = sb.tile([100, 50], f32, tag="pit", name="pit")
            V.tensor_scalar(out=pit, in0=IO50, scalar1=rank, scalar2=None, op0=Alu.is_equal)
            # srank[n] = #kept among m<n  ->  one matmul with strict-upper const
            srank_p = mm([100, 1], "srank_p")
            T.matmul(srank_p, TRIU, kept)
            gat = sb.tile([100, 50], f32, tag="gat", name="gat")
            V.scalar_tensor_tensor(out=gat, in0=IO50, scalar=srank_p, in1=kept.broadcast_to([100, 50]),
                                   op0=Alu.is_equal, op1=Alu.mult)

            # ---- pooled adjacency Atil = Pi @ A1 @ Pi^T --------------------
            m1 = mm([100, 50], "m1")
            T.matmul(m1, a1t, pit)
            m1s = sb.tile([100, 50], f32, tag="m1s", name="m1s")
            V.tensor_copy(out=m1s, in_=m1)
            atil = mm([50, 50], "atil")
            T.matmul(atil, pit, m1s)          # Atil[i,j]
            atilt_p = mm([50, 50], "atilt_p")
            T.matmul(atilt_p, m1s, pit)       # Atil^T[j,i]
            degc = sb.tile([50, 1], f32, tag="degc", name="degc")
            V.tensor_reduce(out=degc, in_=atil, axis=AxX, op=Alu.add)

            # disč = where(deg>0, rsqrt(max(deg,1e-12)), 0)
            dm = sb.tile([50, 1], f32, tag="dm", name="dm")
            V.tensor_scalar(out=dm, in0=degc, scalar1=1e-12, scalar2=None, op0=Alu.max)
            S.activation(out=dm, in_=dm, func=Act.Sqrt)
            V.reciprocal(out=dm, in_=dm)
            m0 = sb.tile([50, 1], f32, tag="m0", name="m0")
            V.tensor_scalar(out=m0, in0=degc, scalar1=0.0, scalar2=None, op0=Alu.is_gt)
            disch = sb.tile([50, 1], f32, tag="disch", name="disch")
            V.tensor_tensor(out=disch, in0=dm, in1=m0, op=Alu.mult)
            # extended to 100 rows (0 beyond 50) so Tx ops run at partition 0
            dise = sb.tile([100, 1], f32, tag="dise", name="dise")
            V.memset(dise, 0.0)
            V.tensor_copy(out=dise[0:50, :], in_=disch)
            ndis = sb.tile([100, 1], f32, tag="ndis", name="ndis")
            V.tensor_scalar_mul(ndis, dise, -1.0)
            n2dis = sb.tile([100, 1], f32, tag="n2dis", name="n2dis")
            V.tensor_scalar_mul(n2dis, dise, -2.0)
            # Atil^T padded to [50,100] so matmul M=100 (rows >=50 produce 0)
            atx = sb.tile([50, 100], f32, tag="atx", name="atx")
            V.memset(atx, 0.0)
            V.tensor_copy(out=atx[:, 0:50], in_=atilt_p)

            # ---- Cheb Tx1 / Tx2 --------------------------------------------
            y1c = sb.tile([50, 20], f32, tag="y1c", name="y1c")
            V.tensor_scalar_mul(y1c, h2[0:50, :], disch)
            tx1p = mm([100, 20], "tx1p")
            T.matmul(tx1p, atx, y1c)
            tx1f = sb.tile([100, 20], f32, tag="tx1f", name="tx1f")
            V.tensor_scalar_mul(tx1f, tx1p, ndis)      # rows>=50 -> 0
            y2c = sb.tile([50, 20], f32, tag="y2c", name="y2c")
            V.tensor_scalar_mul(y2c, tx1f[0:50, :], disch)
            tx2p = mm([100, 20], "tx2p")
            T.matmul(tx2p, atx, y2c)
            tx2f = sb.tile([100, 20], f32, tag="tx2f", name="tx2f")
            # rows<50: -2dis*t - h2 ; rows>=50: 0 - h2  (= -Tx0, as required)
            V.scalar_tensor_tensor(out=tx2f, in0=tx2p, scalar=n2dis, in1=h2,
                                   op0=Alu.mult, op1=Alu.subtract)

            # ---- s_raw = h2@Wc0 + Tx1@Wc1 + Tx2@Wc2 + bc --------------------
            sraw_p = mm([100, 20], "sraw_p")
            T.matmul(sraw_p, h2t, WC0, start=True, stop=False)
            for i, (tq, wc) in enumerate(((tx1f, WC1), (tx2f, WC2))):
                tq_tp = mm([20, 100], f"tq_tp{i}")
                T.transpose(tq_tp, tq, I100)
                tq_ts = sb.tile([20, 100], f32, tag=f"tqts{i}", name=f"tqts{i}")
                V.tensor_copy(out=tq_ts, in_=tq_tp)
                T.matmul(sraw_p, tq_ts, wc, start=False, stop=(i == 1))
            sraw = sb.tile([100, 20], f32, tag="sraw", name="sraw")
            V.tensor_tensor(out=sraw, in0=sraw_p, in1=BCC, op=Alu.add)

            # ---- pooled rows (ready as soon as pit is; overlaps the rest) ---
            p1 = mm([50, 21], "p1")
            T.matmul(p1, pit, h2x[:, 0:21])   # [h2 | score][perm]
            th = sb.tile([50, 1], f32, tag="th", name="th")
            S.activation(out=th, in_=p1[:, 20:21], func=Act.Tanh, bias=BREL[0:50, :], scale=1.0)
            p1s = sb.tile([50, 20], f32, tag="p1s", name="p1s")
            V.tensor_copy(out=p1s, in_=p1[:, 0:20])

            # ---- double softmax; normalizations folded into consumers -------
            # ass = E * recip (never materialized): E=exp(sraw), row sums via
            # ACT accum_out; second exp folds the 1/S scale into the ACT op.
            ex1 = sb.tile([100, 20], f32, tag="ex1", name="ex1")
            sum1 = sb.tile([100, 1], f32, tag="sum1", name="sum1")
            S.activation(out=ex1, in_=sraw, func=Act.Exp, accum_out=sum1)
            rc1 = sb.tile([100, 1], f32, tag="rc1", name="rc1")
            V.reciprocal(out=rc1, in_=sum1)
            ex2 = sb.tile([100, 20], f32, tag="ex2", name="ex2")
            sum2 = sb.tile([100, 1], f32, tag="sum2", name="sum2")
            S.activation(out=ex2, in_=ex1, func=Act.Exp, scale=rc1, accum_out=sum2)
            rc2 = sb.tile([100, 1], f32, tag="rc2", name="rc2")
            V.reciprocal(out=rc2, in_=sum2)
            s2 = sb.tile([100, 20], f32, tag="s2", name="s2")
            V.tensor_scalar_mul(s2, ex2, rc2)

            # ---- diff-pool + output ----------------------------------------
            hc_p = mm([20, 20], "hc_p")
            T.matmul(hc_p, s2, h2)            # H_coarse = s2^T @ h2
            hc = sb.tile([20, 20], f32, tag="hc", name="hc")
            V.tensor_copy(out=hc, in_=hc_p)
            ext_p = mm([20, 100], "ext_p")
            T.transpose(ext_p, ex1, I100)
            ext = sb.tile([20, 100], f32, tag="ext", name="ext")
            V.tensor_copy(out=ext, in_=ext_p)
            ehc = mm([100, 20], "ehc")
            T.matmul(ehc, ext, hc)            # E @ H_coarse
            ehcs = sb.tile([100, 20], f32, tag="ehcs", name="ehcs")
            V.tensor_copy(out=ehcs, in_=ehc)
            gat_r = sb.tile([100, 50], f32, tag="gat_r", name="gat_r")
            V.tensor_scalar_mul(gat_r, gat, rc1)   # fold ass = E/S into Gamma
            g_p = mm([50, 20], "g_p")
            T.matmul(g_p, gat_r, ehcs)        # inter @ H_coarse (rows perm order)
            outv = sb.tile([50, 20], f32, tag="outv", name="outv")
            V.scalar_tensor_tensor(out=outv, in0=p1s, scalar=th, in1=g_p, op0=Alu.mult, op1=Alu.add)
            nc.sync.dma_start(out=out_d.ap(), in_=outv)

    # walrus single-wait workaround
    orig = nc.to_json_bytes
    def patched(*a, **k):
        import json as _json
        return _json.dumps(_split_multiwaits(_json.loads(orig(*a, **k)))).encode()
    nc.to_json_bytes = patched
    return nc


def _pack(inputs) -> np.ndarray:
    f = lambda k: np.asarray(inputs[k], dtype=np.float32)
    blob = np.zeros((128, C_COLS), dtype=np.float32)

    x = f("x")
    blob[0:100, O_XT:O_XT + 100] = x.T

    ei = np.asarray(inputs["edge_index"]).astype(np.int64)
    src = np.full(EP, -1.0, np.float32); src[:E] = ei[0]
    dst = np.full(EP, -1.0, np.float32); dst[:E] = ei[1]
    ew = np.zeros(EP, np.float32); ew[:E] = f("edge_attr")
    # column-chunk layout: element (p, c) = edge c*128+p
    blob[:, O_SRC:O_SRC + 16] = src.reshape(NCH, 128).T
    blob[:, O_DST:O_DST + 16] = dst.reshape(NCH, 128).T
    blob[:, O_EW:O_EW + 16] = ew.reshape(NCH, 128).T

    blob[0:100, O_W1:O_W1 + 64] = f("Wl1")
    blob[0:100, O_W1 + 64:O_W1 + 128] = f("Wr1")
    blob[0:64, O_W2:O_W2 + 20] = f("Wl2")
    blob[0:64, O_W2 + 20:O_W2 + 40] = f("Wr2")
    blob[0:20, O_WG:O_WG + 20] = f("Wg1")
    blob[0:20, O_WREL] = f("Wrel")[:, 0]
    blob[0:20, O_WROOT] = f("Wroot")[:, 0]
    blob[0:20, O_WC:O_WC + 20] = f("Wc0")
    blob[0:20, O_WC + 20:O_WC + 40] = f("Wc1")
    blob[0:20, O_WC + 40:O_WC + 60] = f("Wc2")
    blob[0:50, O_BC1:O_BC1 + 64] = f("bl1")
    blob[50:100, O_BC1:O_BC1 + 64] = f("br1")
    blob[0:50, O_BC2:O_BC2 + 20] = f("bl2")
    blob[50:100, O_BC2:O_BC2 + 20] = f("br2")
    blob[0:100, O_BG:O_BG + 20] = f("bg1")
    blob[0:100, O_BCC:O_BCC + 20] = f("bc")
    blob[:, O_BREL] = f("brel")[0]
    blob[0:50, O_MKL] = 1.0
    blob[50:100, O_MKR] = 1.0
    half = np.arange(100) < 50
    blob[0:100, O_MBD:O_MBD + 100] = (half[:, None] == half[None, :]).astype(np.float32)
    return blob


_NC = None

def _get_nc():
    global _NC
    if _NC is None:
        _NC = _build()
    return _NC


def run(inputs, trace=False):
    from concourse.bass_utils import run_bass_kernel_spmd
    nc = _get_nc()
    blob = _pack(inputs)
    parts = {
        "inbufA": np.ascontiguousarray(blob[:, 0:C_DMA_A]),
        "inbufB": np.ascontiguousarray(blob[:, C_DMA_A:C_DMA_B]),
        "inbufC": np.ascontiguousarray(blob[:, C_DMA_B:C_COLS]),
    }
    in_maps = [dict(parts) for _ in range(8)]
    res = run_bass_kernel_spmd(nc, in_maps, list(range(8)), trace=trace)
    out = np.asarray(res.results[0]["out"], dtype=np.float32).reshape(1, K1 * 20)
    return out, res


def kernel(**inputs) -> np.ndarray:
    out, _ = run(inputs)
    return out


# revision 42
# speedup vs baseline: 1.2496x; 1.0082x over previous
"""Trainium2 Bass kernel for nn_Brain_connectomic_graph (GNN message passing).

Single tiny graph (N=100 nodes, E=2000 edges). Strategy: the whole network is
expressed as dense linear algebra on ONE NeuronCore and replicated across the
8 cores (data-parallel lanes with batch=1, per the sharding hint); core 0's
output is returned.

All floating-point math runs on device. The host only does layout packing:
  - transposes/concats of input tensors (pure data movement),
  - integer edge indices packed as f32 columns (one-hot encoding happens
    on-device via iota comparison),
  - pure constants (iota rows, triangular masks, identity, ones).

Graph ops are densified on device:
  - scatter-adds over edges -> one-hot matrices (DVE compares, pipelined in
    4 chunk-groups with the weighted variants on GpSimd) contracted on the
    PE: A^T stacked for (unweighted | same-hemisphere | full weighted),
  - GCN normalization  -> row-scaling sandwich dis * ((A+I)^T' @ (dis * XW)),
  - top-k(50)         -> rank via score comparison matrix (strict > plus
    index tie-break, matching jax.lax.top_k), permutation as one-hot matmul,
  - SAGPool / ChebConv / dense_diff_pool -> small matmuls + softmaxes.
"""

import numpy as np

N = 100
E = 2000
EP = 2048          # padded edges: 16 chunks x 128 partitions
NCH = 16
K1 = 50

# ---- inbuf column layout (f32 blob [128, C]) --------------------------------
# Ordered by when the kernel needs the data; loaded as 3 parallel DMAs.
_off = 0
def _nxt(w):
    global _off
    o = _off
    _off += w
    return o

# DMA group A (own DRAM tensor, contiguous): edge data
O_SRC   = _nxt(16)    # [128,16]  src (f32, pad -1)
O_DST   = _nxt(16)    # [128,16]  dst (f32, pad -1)
O_EW    = _nxt(16)    # [128,16]  edge_attr (pad 0)
C_DMA_A = _off
# DMA group B: first matmul operands
O_XT    = _nxt(100)   # [100,100] x^T
O_W1    = _nxt(128)   # [100,128] [Wl1 | Wr1]
C_DMA_B = _off
# DMA group C: everything else
O_W2    = _nxt(40)    # [64,40]   [Wl2 | Wr2]
O_WG    = _nxt(20)    # [20,20]   Wg1
O_WREL  = _nxt(1)     # [20,1]    Wrel
O_WROOT = _nxt(1)     # [20,1]    Wroot
O_WC    = _nxt(60)    # [20,60]   [Wc0 | Wc1 | Wc2]
O_BC1   = _nxt(64)    # [100,64]  rows<50: bl1, rows>=50: br1
O_BC2   = _nxt(20)    # [100,20]  rows<50: bl2, rows>=50: br2
O_BG    = _nxt(20)    # [100,20]  bg1 broadcast
O_BCC   = _nxt(20)    # [100,20]  bc broadcast
O_BREL  = _nxt(1)     # [128,1]   brel broadcast
O_MKL   = _nxt(1)     # [128,1]   1.0 for p<50 else 0
O_MKR   = _nxt(1)     # [128,1]   1.0 for 50<=p<100 else 0
O_MBD   = _nxt(100)   # [100,100] block mask: [b,a]=1 iff (b<50)==(a<50)
C_COLS  = _off
# Pure constants (iota / identity / tril / triu / ones) are generated
# on-device by GpSimd during the DMA window.


def _split_multiwaits(bir: dict) -> dict:
    """This container's walrus accepts only ONE sync-wait per instruction.
    Insert single-wait NoOps (same engine, just before) for the extras."""
    for f in bir.get("functions", []):
        for bb in f.get("blocks", []):
            out = []
            for ins in bb.get("instructions", []):
                si = ins.get("sync_info")
                waits = (si or {}).get("on_wait") or []
                if len(waits) > 1:
                    for i, w in enumerate(waits[:-1]):
                        out.append({
                            "debug": ins.get("debug", 0),
                            "engine": ins["engine"],
                            "ins": [], "outs": [],
                            "name": f"{ins['name']}-w{i}",
                            "opcode": "NoOp",
                            "sync_info": {"on_wait": [w], "on_update": []},
                        })
                    si["on_wait"] = [waits[-1]]
                out.append(ins)
            bb["instructions"] = out
    return bir


def _build():
    import concourse.bass as bass
    import concourse.mybir as mybir
    import concourse.tile as tile

    f32 = mybir.dt.float32
    Alu = mybir.AluOpType
    Act = mybir.ActivationFunctionType
    AxX = mybir.AxisListType.X

    nc = bass.Bass("TRN2")
    in_a = nc.dram_tensor("inbufA", [128, C_DMA_A], f32, kind="ExternalInput")
    in_b = nc.dram_tensor("inbufB", [128, C_DMA_B - C_DMA_A], f32, kind="ExternalInput")
    in_c = nc.dram_tensor("inbufC", [128, C_COLS - C_DMA_B], f32, kind="ExternalInput")
    out_d = nc.dram_tensor("out", [K1, 20], f32, kind="ExternalOutput")

    with tile.TileContext(nc) as tc:
        with (
            tc.tile_pool(name="sb", bufs=1) as sb,
            tc.tile_pool(name="ps", bufs=1, space="PSUM") as ps,
        ):
            ib = sb.tile([128, C_COLS], f32, tag="ib", name="ib")
            nc.sync.dma_start(out=ib[:, 0:C_DMA_A], in_=in_a.ap())
            nc.sync.dma_start(out=ib[:, C_DMA_A:C_DMA_B], in_=in_b.ap())
            nc.sync.dma_start(out=ib[:, C_DMA_B:C_COLS], in_=in_c.ap())

            def isl(off, w, p0=0, p1=128):
                return ib[p0:p1, off:off + w]

            # ---- on-device constants (GpSimd, runs during the DMAs) ---------
            iota_i = sb.tile([128, 100], mybir.dt.int32, tag="iota_i", name="iota_i")
            nc.gpsimd.iota(iota_i, pattern=[[1, 100]], base=0, channel_multiplier=0)
            iota_t = sb.tile([128, 100], f32, tag="iota_t", name="iota_t")
            nc.gpsimd.tensor_copy(out=iota_t, in_=iota_i)
            i100_t = sb.tile([100, 100], f32, tag="i100_t", name="i100_t")
            nc.gpsimd.memset(i100_t, 0.0)
            nc.gpsimd.affine_select(out=i100_t, in_=i100_t, compare_op=mybir.AluOpType.not_equal,
                                    fill=1.0, base=0, pattern=[[-1, 100]], channel_multiplier=1)
            tril_t = sb.tile([100, 100], f32, tag="tril_t", name="tril_t")
            nc.gpsimd.memset(tril_t, 1.0)
            nc.gpsimd.affine_select(out=tril_t, in_=tril_t, compare_op=mybir.AluOpType.is_gt,
                                    fill=0.0, base=0, pattern=[[-1, 100]], channel_multiplier=1)
            triu_t = sb.tile([100, 100], f32, tag="triu_t", name="triu_t")
            nc.gpsimd.memset(triu_t, 1.0)
            nc.gpsimd.affine_select(out=triu_t, in_=triu_t, compare_op=mybir.AluOpType.is_gt,
                                    fill=0.0, base=0, pattern=[[1, 100]], channel_multiplier=-1)
            ones_t = sb.tile([128, 100], f32, tag="ones_t", name="ones_t")
            nc.gpsimd.memset(ones_t, 1.0)

            XT   = isl(O_XT, 100, 0, 100)
            SRC  = isl(O_SRC, 16)
            DST  = isl(O_DST, 16)
            EW   = isl(O_EW, 16)
            W1   = isl(O_W1, 128, 0, 100)
            W2   = isl(O_W2, 40, 0, 64)
            WG   = isl(O_WG, 20, 0, 20)
            WRR2 = isl(O_WREL, 2, 0, 20)      # [Wrel | Wroot]
            WC0  = isl(O_WC, 20, 0, 20)
            WC1  = isl(O_WC + 20, 20, 0, 20)
            WC2  = isl(O_WC + 40, 20, 0, 20)
            BC1  = isl(O_BC1, 64, 0, 100)
            BC2  = isl(O_BC2, 20, 0, 100)
            BG   = isl(O_BG, 20, 0, 100)
            BCC  = isl(O_BCC, 20, 0, 100)
            BREL = isl(O_BREL, 1)
            MKL  = isl(O_MKL, 1, 0, 100)
            MKR  = isl(O_MKR, 1, 0, 100)
            MBD  = isl(O_MBD, 100, 0, 100)
            IOTA = iota_t[:, :]
            IO50 = iota_t[0:100, 0:50]
            TRIL = tril_t[:, :]
            TRIU = triu_t[:, :]
            I100 = i100_t[:, :]
            ONESR = ones_t[0:1, :]             # [1,100] ones row
            ONESC = ones_t[0:100, 0:1]         # [100,1] ones col

            V = nc.vector
            S = nc.scalar
            P = nc.gpsimd
            T = nc.tensor
            mm = lambda shape, name: ps.tile(shape, f32, tag="mm", name=name, bufs=3)

            # ---- ACT table prewarm (Exp/Tanh tables load during prologue) ---
            scr = sb.tile([1, 1], f32, tag="scr", name="scr")
            V.memset(scr, 0.0)
            S.activation(out=scr, in_=scr, func=Act.Exp)
            S.activation(out=scr, in_=scr, func=Act.Tanh)
            S.activation(out=scr, in_=scr, func=Act.Sqrt)

            # ---- PE warmup: dummy matmuls on ones (HAM needs ~4us busy),
            # then xw (only needs DMA group B) --------------------------------
            warm = ps.tile([100, 300], f32, tag="warm", name="warm", bufs=1)
            ones_w = ones_t[:, :].unsqueeze(1).broadcast_to([128, 3, 100])
            for _ in range(4):
                T.matmul(warm, ones_t[:, :], ones_w)
            xw = mm([100, 128], "xw")
            T.matmul(xw, XT, W1)

            # ---- one-hot edge matrices, pipelined in 4 chunk-groups --------
            # Ssrc[e,n] = [src_e == n]; R = [Sdst | Sdst*ew].
            # A_c (same-hemisphere) is NOT built from edges: it equals the
            # block mask applied to A_g, so only 3 one-hot tensors are needed.
            ssrc = sb.tile([128, NCH * 100], f32, tag="ssrc", name="ssrc")
            rall = sb.tile([128, NCH * 200], f32, tag="rall", name="rall")
            ssrc3 = ssrc.rearrange("p (c j) -> p c j", c=NCH)
            rall3 = rall.rearrange("p (c j) -> p c j", c=NCH)
            a_ps = ps.tile([100, 200], f32, tag="acc", name="a_ps", bufs=1)
            GRP = 4
            for g in range(0, NCH, GRP):
                gs_, ge_ = g, g + GRP
                iota_b = IOTA.unsqueeze(1).broadcast_to([128, GRP, 100])
                src_b = SRC[:, gs_:ge_].unsqueeze(2).broadcast_to([128, GRP, 100])
                dst_b = DST[:, gs_:ge_].unsqueeze(2).broadcast_to([128, GRP, 100])
                ew_b = EW[:, gs_:ge_].unsqueeze(2).broadcast_to([128, GRP, 100])
                V.tensor_tensor(out=rall3[:, gs_:ge_, 0:100], in0=iota_b, in1=dst_b, op=Alu.is_equal)
                V.tensor_tensor(out=ssrc3[:, gs_:ge_, 0:100], in0=iota_b, in1=src_b, op=Alu.is_equal)
                P.tensor_tensor(out=rall3[:, gs_:ge_, 100:200], in0=rall3[:, gs_:ge_, 0:100], in1=ew_b, op=Alu.mult)
                for c in range(gs_, ge_):
                    T.matmul(a_ps, ssrc3[:, c, :], rall3[:, c, :],
                             start=(c == 0), stop=(c == NCH - 1))

            a1t = sb.tile([100, 100], f32, tag="a1t", name="a1t")
            act = sb.tile([100, 100], f32, tag="act", name="act")
            agt = sb.tile([100, 100], f32, tag="agt", name="agt")
            V.tensor_copy(out=a1t, in_=a_ps[:, 0:100])
            V.tensor_tensor(out=agt, in0=a_ps[:, 100:200], in1=I100, op=Alu.add)
            # A has no self-loops and diag(MBD)=1, so (A_g+I) masked == A_c+I
            V.tensor_tensor(out=act, in0=agt, in1=MBD, op=Alu.mult)

            # ---- degrees + dis (GCN: deg+1 = rowsum(A+I)) -------------------
            d1c = mm([100, 1], "d1c")
            T.matmul(d1c, act, ONESC)
            d1g = mm([100, 1], "d1g")
            T.matmul(d1g, agt, ONESC)
            disc_t = sb.tile([100, 1], f32, tag="disc", name="disc_t")
            disg_t = sb.tile([100, 1], f32, tag="disg", name="disg_t")
            S.activation(out=disc_t, in_=d1c, func=Act.Sqrt)
            V.reciprocal(out=disc_t, in_=disc_t)
            S.activation(out=disg_t, in_=d1g, func=Act.Sqrt)
            V.reciprocal(out=disg_t, in_=disg_t)
            # ---- layer 1: h1 = lrelu(dis*((Ac+I)^T' @ (dis*xw_side)) + b) ---
            # hemisphere select (masks are 0/1: exact) runs BEFORE dis is
            # ready, so only one scale op sits on the critical path after it
            y1m = sb.tile([100, 64], f32, tag="y1m", name="y1m")
            V.tensor_scalar_mul(y1m, xw[:, 64:128], MKR)
            V.scalar_tensor_tensor(out=y1m, in0=xw[:, 0:64], scalar=MKL, in1=y1m,
                                   op0=Alu.mult, op1=Alu.add)
            y1 = sb.tile([100, 64], f32, tag="y1", name="y1")
            V.tensor_scalar_mul(y1, y1m, disc_t)
            z1 = mm([100, 64], "z1")
            T.matmul(z1, act, y1)
            h1 = sb.tile([100, 64], f32, tag="h1", name="h1")
            V.scalar_tensor_tensor(out=h1, in0=z1, scalar=disc_t, in1=BC1, op0=Alu.mult, op1=Alu.add)
            V.scalar_tensor_tensor(out=h1, in0=h1, scalar=0.01, in1=h1, op0=Alu.mult, op1=Alu.max)

            # ---- layer 2 ----------------------------------------------------
            h1t_p = mm([64, 100], "h1t_p")
            T.transpose(h1t_p, h1, I100)
            h1t = sb.tile([64, 100], f32, tag="h1t", name="h1t")
            V.tensor_copy(out=h1t, in_=h1t_p)
            xw2 = mm([100, 40], "xw2")
            T.matmul(xw2, h1t, W2)
            y2m = sb.tile([100, 20], f32, tag="y2m", name="y2m")
            V.tensor_scalar_mul(y2m, xw2[:, 20:40], MKR)
            V.scalar_tensor_tensor(out=y2m, in0=xw2[:, 0:20], scalar=MKL, in1=y2m,
                                   op0=Alu.mult, op1=Alu.add)
            y2 = sb.tile([100, 20], f32, tag="y2", name="y2")
            V.tensor_scalar_mul(y2, y2m, disc_t)
            z2 = mm([100, 20], "z2")
            T.matmul(z2, act, y2)
            h2a = sb.tile([100, 20], f32, tag="h2a", name="h2a")
            V.scalar_tensor_tensor(out=h2a, in0=z2, scalar=disc_t, in1=BC2, op0=Alu.mult, op1=Alu.add)
            V.scalar_tensor_tensor(out=h2a, in0=h2a, scalar=0.01, in1=h2a, op0=Alu.mult, op1=Alu.max)

            # ---- global GCN layer ------------------------------------------
            h2at_p = mm([20, 100], "h2at_p")
            T.transpose(h2at_p, h2a, I100)
            h2at = sb.tile([20, 100], f32, tag="h2at", name="h2at")
            V.tensor_copy(out=h2at, in_=h2at_p)
            xwg = mm([100, 20], "xwg")
            T.matmul(xwg, h2at, WG)
            yg = sb.tile([100, 20], f32, tag="yg", name="yg")
            V.tensor_scalar_mul(yg, xwg, disg_t)
            zg = mm([100, 20], "zg")
            T.matmul(zg, agt, yg)
            # h2 lives in cols 0:20 of h2x; the SAG score joins as col 20 so
            # one matmul later produces both h2[perm] and score[perm].
            h2x = sb.tile([100, 21], f32, tag="h2x", name="h2x")
            h2 = h2x[:, 0:20]
            score = h2x[:, 20:21]
            V.scalar_tensor_tensor(out=h2, in0=zg, scalar=disg_t, in1=BG, op0=Alu.mult, op1=Alu.add)
            V.scalar_tensor_tensor(out=h2, in0=h2, scalar=0.01, in1=h2, op0=Alu.mult, op1=Alu.max)
            # h2^T, reused by the score matmuls and s_raw stage
            h2t_p = mm([20, 100], "h2t_p")
            T.transpose(h2t_p, h2, I100)
            h2t = sb.tile([20, 100], f32, tag="h2t", name="h2t")
            V.tensor_copy(out=h2t, in_=h2t_p)

            # ---- SAGPool score = A1@(h2@Wrel) + h2@Wroot  (brel in tanh) ----
            hw = mm([100, 2], "hw")
            T.matmul(hw, h2t, WRR2)           # [h2@Wrel | h2@Wroot]
            hw_sb = sb.tile([100, 2], f32, tag="hw_sb", name="hw_sb")
            V.tensor_copy(out=hw_sb, in_=hw)
            sc_p = mm([100, 1], "sc_p")
            T.matmul(sc_p, a1t, hw_sb[:, 0:1])
            V.tensor_tensor(out=score, in0=sc_p, in1=hw_sb[:, 1:2], op=Alu.add)

            # ---- rank / top-k as matrices ----------------------------------
            # score row MUST be bit-identical to the score column (the rank
            # comparisons mix both); a PE transpose preserves bits, a separate
            # matmul accumulation order does not.
            srow_p = mm([1, 100], "srow_p")
            T.transpose(srow_p, score, I100)
            srow = sb.tile([1, 100], f32, tag="srow", name="srow")
            V.tensor_copy(out=srow, in_=srow_p)
            srep = ps.tile([100, 100], f32, tag="rep", name="srep", bufs=1)
            T.matmul(srep, ONESR, srow)       # srep[n,m] = score[m]
            t2 = sb.tile([100, 100], f32, tag="t2", name="t2")
            V.scalar_tensor_tensor(out=t2, in0=srep, scalar=score, in1=TRIL, op0=Alu.is_equal, op1=Alu.mult)
            csum = sb.tile([100, 100], f32, tag="csum", name="csum")
            rank = sb.tile([100, 1], f32, tag="rank", name="rank")
            V.scalar_tensor_tensor(out=csum, in0=srep, scalar=score, in1=t2, op0=Alu.is_gt, op1=Alu.add,
                                   accum_out=rank)
            kept = sb.tile([100, 1], f32, tag="kept", name="kept")
            V.tensor_scalar(out=kept, in0=rank, scalar1=49.5, scalar2=None, op0=Alu.is_lt)
            pit = sb.tile([100, 50], f32, tag="pit", name="pit")
            V.tensor_scalar(out=pit, in0=IO50, scalar1=rank, scalar2=None, op0=Alu.is_equal)
            # srank[n] = #kept among m<n  ->  one matmul with strict-upper const
            srank_p = mm([100, 1], "srank_p")
            T.matmul(srank_p, TRIU, kept)
            gat = sb.tile([100, 50], f32, tag="gat", name="gat")
            V.scalar_tensor_tensor(out=gat, in0=IO50, scalar=srank_p, in1=kept.broadcast_to([100, 50]),
                                   op0=Alu.is_equal, op1=Alu.mult)

            # ---- pooled adjacency Atil = Pi @ A1 @ Pi^T --------------------
            m1 = mm([100, 50], "m1")
            T.matmul(m1, a1t, pit)
            m1s = sb.tile([100, 50], f32, tag="m1s", name="m1s")
            V.tensor_copy(out=m1s, in_=m1)
            atil = mm([50, 50], "atil")
            T.matmul(atil, pit, m1s)          # Atil[i,j]
            atilt_p = mm([50, 50], "atilt_p")
            T.matmul(atilt_p, m1s, pit)       # Atil^T[j,i]
            degc = sb.tile([50, 1], f32, tag="degc", name="degc")
            V.tensor_reduce(out=degc, in_=atil, axis=AxX, op=Alu.add)

            # disč = where(deg>0, rsqrt(max(deg,1e-12)), 0)
            dm = sb.tile([50, 1], f32, tag="dm", name="dm")
            V.tensor_scalar(out=dm, in0=degc, scalar1=1e-12, scalar2=None, op0=Alu.max)
            S.activation(out=dm, in_=dm, func=Act.Sqrt)
            V.reciprocal(out=dm, in_=dm)
            m0 = sb.tile([50, 1], f32, tag="m0", name="m0")
            V.tensor_scalar(out=m0, in0=degc, scalar1=0.0, scalar2=None, op0=Alu.is_gt)
            disch = sb.tile([50, 1], f32, tag="disch", name="disch")
            V.tensor_tensor(out=disch, in0=dm, in1=m0, op=Alu.mult)
            # extended to 100 rows (0 beyond 50) so Tx ops run at partition 0
            dise = sb.tile([100, 1], f32, tag="dise", name="dise")
            V.memset(dise, 0.0)
            V.tensor_copy(out=dise[0:50, :], in_=disch)
            ndis = sb.tile([100, 1], f32, tag="ndis", name="ndis")
            V.tensor_scalar_mul(ndis, dise, -1.0)
            n2dis = sb.tile([100, 1], f32, tag="n2dis", name="n2dis")
            V.tensor_scalar_mul(n2dis, dise, -2.0)
            # Atil^T padded to [50,100] so matmul M=100 (rows >=50 produce 0)
            atx = sb.tile([50, 100], f32, tag="atx", name="atx")
            V.memset(atx, 0.0)
            V.tensor_copy(out=atx[:, 0:50], in_=atilt_p)

            # ---- Cheb Tx1 / Tx2 --------------------------------------------
            y1c = sb.tile([50, 20], f32, tag="y1c", name="y1c")
            V.tensor_scalar_mul(y1c, h2[0:50, :], disch)
            tx1p = mm([100, 20], "tx1p")
            T.matmul(tx1p, atx, y1c)
            tx1f = sb.tile([100, 20], f32, tag="tx1f", name="tx1f")
            V.tensor_scalar_mul(tx1f, tx1p, ndis)      # rows>=50 -> 0
            y2c = sb.tile([50, 20], f32, tag="y2c", name="y2c")
            V.tensor_scalar_mul(y2c, tx1f[0:50, :], disch)
            tx2p = mm([100, 20], "tx2p")
            T.matmul(tx2p, atx, y2c)
            tx2f = sb.tile([100, 20], f32, tag="tx2f", name="tx2f")
            # rows<50: -2dis*t - h2 ; rows>=50: 0 - h2  (= -Tx0, as required)
            V.scalar_tensor_tensor(out=tx2f, in0=tx2p, scalar=n2dis, in1=h2,
                                   op0=Alu.mult, op1=Alu.subtract)

            # ---- s_raw = h2@Wc0 + Tx1@Wc1 + Tx2@Wc2 + bc --------------------
            sraw_p = mm([100, 20], "sraw_p")
            T.matmul(sraw_p, h2t, WC0, start=True, stop=False)
            for i, (tq, wc) in enumerate(((tx1f, WC1), (tx2f, WC2))):
                tq_tp = mm([20, 100], f"tq_tp{i}")
                T.transpose(tq_tp, tq, I100)
                tq_ts = sb.tile([20, 100], f32, tag=f"tqts{i}", name=f"tqts{i}")
                V.tensor_copy(out=tq_ts, in_=tq_tp)
                T.matmul(sraw_p, tq_ts, wc, start=False, stop=(i == 1))
            sraw = sb.tile([100, 20], f32, tag="sraw", name="sraw")
            V.tensor_tensor(out=sraw, in0=sraw_p, in1=BCC, op=Alu.add)

            # ---- pooled rows (ready as soon as pit is; overlaps the rest) ---
            p1 = mm([50, 21], "p1")
            T.matmul(p1, pit, h2x[:, 0:21])   # [h2 | score][perm]
            th = sb.tile([50, 1], f32, tag="th", name="th")
            S.activation(out=th, in_=p1[:, 20:21], func=Act.Tanh, bias=BREL[0:50, :], scale=1.0)
            p1s = sb.tile([50, 20], f32, tag="p1s", name="p1s")
            V.tensor_copy(out=p1s, in_=p1[:, 0:20])

            # ---- double softmax; normalizations folded into consumers -------
            # ass = E * recip (never materialized): E=exp(sraw), row sums via
            # ACT accum_out; second exp folds the 1/S scale into the ACT op.
            ex1 = sb.tile([100, 20], f32, tag="ex1", name="ex1")
            sum1 = sb.tile([100, 1], f32, tag="sum1", name="sum1")
            S.activation(out=ex1, in_=sraw, func=Act.Exp, accum_out=sum1)
            rc1 = sb.tile([100, 1], f32, tag="rc1", name="rc1")
            V.reciprocal(out=rc1, in_=sum1)
            ex2 = sb.tile([100, 20], f32, tag="ex2", name="ex2")
            sum2 = sb.tile([100, 1], f32, tag="sum2", name="sum2")
            S.activation(out=ex2, in_=ex1, func=Act.Exp, scale=rc1, accum_out=sum2)
            rc2 = sb.tile([100, 1], f32, tag="rc2", name="rc2")
            V.reciprocal(out=rc2, in_=sum2)
            s2 = sb.tile([100, 20], f32, tag="s2", name="s2")
            V.tensor_scalar_mul(s2, ex2, rc2)

            # ---- diff-pool + output ----------------------------------------
            hc_p = mm([20, 20], "hc_p")
            T.matmul(hc_p, s2, h2)            # H_coarse = s2^T @ h2
            hc = sb.tile([20, 20], f32, tag="hc", name="hc")
            V.tensor_copy(out=hc, in_=hc_p)
            ext_p = mm([20, 100], "ext_p")
            T.transpose(ext_p, ex1, I100)
            ext = sb.tile([20, 100], f32, tag="ext", name="ext")
            V.tensor_copy(out=ext, in_=ext_p)
            ehc = mm([100, 20], "ehc")
            T.matmul(ehc, ext, hc)            # E @ H_coarse
            ehcs = sb.tile([100, 20], f32, tag="ehcs", name="ehcs")
            V.tensor_copy(out=ehcs, in_=ehc)
            gat_r = sb.tile([100, 50], f32, tag="gat_r", name="gat_r")
            V.tensor_scalar_mul(gat_r, gat, rc1)   # fold ass = E/S into Gamma
            g_p = mm([50, 20], "g_p")
            T.matmul(g_p, gat_r, ehcs)        # inter @ H_coarse (rows perm order)
            outv = sb.tile([50, 20], f32, tag="outv", name="outv")
            V.scalar_tensor_tensor(out=outv, in0=p1s, scalar=th, in1=g_p, op0=Alu.mult, op1=Alu.add)
            nc.sync.dma_start(out=out_d.ap(), in_=outv)

    # walrus single-wait workaround
    orig = nc.to_json_bytes
    def patched(*a, **k):
        import json as _json
        return _json.dumps(_split_multiwaits(_json.loads(orig(*a, **k)))).encode()
    nc.to_json_bytes = patched
    return nc


def _pack(inputs) -> np.ndarray:
    f = lambda k: np.asarray(inputs[k], dtype=np.float32)
    blob = np.zeros((128, C_COLS), dtype=np.float32)

    x = f("x")
    blob[0:100, O_XT:O_XT + 100] = x.T

    ei = np.asarray(inputs["edge_index"]).astype(np.int64)
    src = np.full(EP, -1.0, np.float32); src[:E] = ei[0]
    dst = np.full(EP, -1.0, np.float32); dst[:E] = ei[1]
    ew = np.zeros(EP, np.float32); ew[:E] = f("edge_attr")
    # column-chunk layout: element (p, c) = edge c*128+p
    blob[:, O_SRC:O_SRC + 16] = src.reshape(NCH, 128).T
    blob[:, O_DST:O_DST + 16] = dst.reshape(NCH, 128).T
    blob[:, O_EW:O_EW + 16] = ew.reshape(NCH, 128).T

    blob[0:100, O_W1:O_W1 + 64] = f("Wl1")
    blob[0:100, O_W1 + 64:O_W1 + 128] = f("Wr1")
    blob[0:64, O_W2:O_W2 + 20] = f("Wl2")
    blob[0:64, O_W2 + 20:O_W2 + 40] = f("Wr2")
    blob[0:20, O_WG:O_WG + 20] = f("Wg1")
    blob[0:20, O_WREL] = f("Wrel")[:, 0]
    blob[0:20, O_WROOT] = f("Wroot")[:, 0]
    blob[0:20, O_WC:O_WC + 20] = f("Wc0")
    blob[0:20, O_WC + 20:O_WC + 40] = f("Wc1")
    blob[0:20, O_WC + 40:O_WC + 60] = f("Wc2")
    blob[0:50, O_BC1:O_BC1 + 64] = f("bl1")
    blob[50:100, O_BC1:O_BC1 + 64] = f("br1")
    blob[0:50, O_BC2:O_BC2 + 20] = f("bl2")
    blob[50:100, O_BC2:O_BC2 + 20] = f("br2")
    blob[0:100, O_BG:O_BG + 20] = f("bg1")
    blob[0:100, O_BCC:O_BCC + 20] = f("bc")
    blob[:, O_BREL] = f("brel")[0]
    blob[0:50, O_MKL] = 1.0
    blob[50:100, O_MKR] = 1.0
    half = np.arange(100) < 50
    blob[0:100, O_MBD:O_MBD + 100] = (half[:, None] == half[None, :]).astype(np.float32)
    return blob


_NC = None

def _get_nc():
    global _NC
    if _NC is None:
        _NC = _build()
    return _NC


def run(inputs, trace=False):
    from concourse.bass_utils import run_bass_kernel_spmd
    nc = _get_nc()
    blob = _pack(inputs)
    parts = {
        "inbufA": np.ascontiguousarray(blob[:, 0:C_DMA_A]),
        "inbufB": np.ascontiguousarray(blob[:, C_DMA_A:C_DMA_B]),
        "inbufC": np.ascontiguousarray(blob[:, C_DMA_B:C_COLS]),
    }
    in_maps = [dict(parts) for _ in range(8)]
    res = run_bass_kernel_spmd(nc, in_maps, list(range(8)), trace=trace)
    out = np.asarray(res.results[0]["out"], dtype=np.float32).reshape(1, K1 * 20)
    return out, res


def kernel(**inputs) -> np.ndarray:
    out, _ = run(inputs)
    return out
